# revision 28
# baseline (speedup 1.0000x reference)
"""Causal self-attention (B=8, T=1024, C=768, H=12) on 8 Trainium2 NeuronCores.

Sharding: data parallel - one batch element per core, no collectives.

v3: all matmuls bf16 (1 cyc/row at any free size, half the DMA/SBUF of
fp32), fp32 accumulation in PSUM.
  Q^T, K^T = Wqkv^T-tiles.T @ x^T       (features on partitions, bf16 out)
  V  = x^T-tiles.T @ Wv (+ ones col)    (natural layout per head)
  per head pair (even head on partitions 0-63, odd on 64-127; the S
  matmuls are 64-contraction row-tiled pairs emitted interleaved so both
  halves of the PE array can run concurrently on hardware):
    S^T strips = K_h^T.T @ Q_h^T        (exact causal chunks; strips for
       kt=2,3 and kt=4..7 share one PSUM tile to cut ACT op count)
    ACT exp(s/8) -> ragged P^T (bf16); no mask matmuls - the invalid
       lower triangle of each diagonal 128x128 block is zeroed after the
       exp by a gpsimd multiply with a 0/1 triangle (exp of junk is
       finite, then exactly zeroed)
    O'^T = [V_h | 1].T @ P^T            (row 64 = softmax denominator)
    normalize: gpsimd partition_broadcast of the raw denominator row
       (read straight from PSUM partition 64) + one DVE divide
       (odd heads: SBUF->SBUF DMA shifts the result to partitions 64-127)
  y = attn^T-tiles.T @ Wp + bias (f32)
S-strip emission is interleaved into phase 1 (after each pair's Q/K
tiles complete) so the scalar engine's exp stream - the phase-2 pacer -
starts ~15us into the kernel. Weight DMAs issue from the gpsimd queue to
unserialize the SP engine.
"""
import sys
from contextlib import ExitStack

import numpy as np

for _p in ("/opt/trn_rl_repo", "/root/.axon_site/_ro/trn_rl_repo"):
    if _p not in sys.path:
        sys.path.insert(0, _p)

import concourse.bass as bass  # noqa: E402
import concourse.mybir as mybir  # noqa: E402

F32 = mybir.dt.float32
BF16 = mybir.dt.bfloat16
AF = mybir.ActivationFunctionType
OP = mybir.AluOpType

B, T, C, H, D = 8, 1024, 768, 12, 64
N_CORES = 8
NT = T // 128   # 8 query/key tiles
NC = C // 128   # 6 feature tiles
NPAIR = H // 2  # 6 head pairs == feature tiles

# ragged P^T offsets: strip kt holds q in [kt*128, T)
OFF = [0] * (NT + 1)
for _i in range(NT):
    OFF[_i + 1] = OFF[_i] + (T - 128 * _i)
PTW = OFF[NT]

# PSUM strip groups: list of (kt members,) merged into one tile / one ACT.
# Max tile 1024 f32 = 2 banks, so strip pool = 4 banks and the O pool can
# quad-buffer.
SGROUPS = [(0,), (1,), (2,), (3,), (4, 5), (6, 7)]


def _chunks_512(a, b):
    out = []
    while a < b:
        nxt = min((a // 512 + 1) * 512, b)
        out.append((a, nxt))
        a = nxt
    return out


def _emit_attention(tc, io):
    nc = tc.nc

    with ExitStack() as stack:
        consts = stack.enter_context(tc.tile_pool(name="consts", bufs=1))
        persist = stack.enter_context(tc.tile_pool(name="persist", bufs=1))

        qt = persist.tile([128, NC, T], BF16, tag="qt")
        kt_ = persist.tile([128, NC, T], BF16, tag="kt")
        vp = persist.tile([128, NT, H, D + 1], BF16, tag="vp")
        attnT = persist.tile([128, NC, T], BF16, tag="attnT")
        wpp = persist.tile([128, NC, C], BF16, tag="wpp")

        # phase-2 long-lived pools first (pool closes must be LIFO, and the
        # phase-1 pools close mid-emission)
        ptp = stack.enter_context(tc.tile_pool(name="pt", bufs=2))
        nrm = stack.enter_context(tc.tile_pool(name="nrm", bufs=3))
        sps_cm = tc.tile_pool(name="sps", bufs=1, space="PSUM")
        sps = sps_cm.__enter__()

        # phase-1 working set (closed mid-emission, after V)
        p1w_cm = tc.tile_pool(name="p1w", bufs=1)
        p1w = p1w_cm.__enter__()
        ps1_cm = tc.tile_pool(name="ps1", bufs=2, space="PSUM")
        ps1 = ps1_cm.__enter__()
        x1t = p1w.tile([128, NC, T], BF16, tag="x1t")
        wq_sb = p1w.tile([128, NC, 3 * C], BF16, tag="wq")

        # x first (gates first matmul), split across SP/DVE queues; weights
        # on the Pool/ACT queues so SP's serial descriptor generation
        # doesn't starve PE
        for c in range(NC):
            for hi, (a, b) in enumerate(((0, 512), (512, 1024))):
                eng = nc.sync if (c + hi) % 2 == 0 else nc.scalar
                eng.dma_start(x1t[:, c, a:b],
                              io["xT"][c * 128:(c + 1) * 128, a:b])
        tri01 = consts.tile([128, 128], BF16, tag="tri01")
        nc.sync.dma_start(tri01[:], io["tri01"])
        bqt = consts.tile([128, 2 * C // 128], F32, tag="bqt")
        nc.sync.dma_start(bqt[:], io["bqkvT"])
        bb = consts.tile([128, 2 * C], F32, tag="bb")
        nc.sync.dma_start(bb[:], io["bias_bcast"])
        ones_sb = consts.tile([128, 128], F32, tag="ones_sb")
        nc.sync.dma_start(ones_sb[:], io["ones"])

        # Q/K weight columns in 256-wide (f, f+NC)-paired groups so the
        # m-order below unblocks in emission order; V columns after
        for g in range(3):
            for base in (0, C):
                c0 = base + g * 256
                for c in range(NC):
                    nc.gpsimd.dma_start(
                        wq_sb[:, c, c0:c0 + 256],
                        io["wqkv"][c * 128:(c + 1) * 128, c0:c0 + 256])
        for c in range(NC):
            nc.gpsimd.dma_start(wq_sb[:, c, 2 * C:3 * C],
                                io["wqkv"][c * 128:(c + 1) * 128, 2 * C:3 * C])
        for c in range(NC):
            nc.sync.dma_start(wpp[:, c, :], io["wp"][c * 128:(c + 1) * 128, :])

        # ones column of V (index 0: the O' matmul then puts the softmax
        # denominator on PSUM partition 0, where DVE/gpsimd can use it
        # without a partition-staging DMA)
        nc.vector.tensor_copy(
            vp[:, :, :, 0],
            ones_sb[:, 0:NT * H].rearrange("p (t h) -> p t h", h=H))

        def emit_qk(f):
            for m in (f, NC + f):
                dest = qt if m < NC else kt_
                for (a, b) in ((0, 512), (512, 1024)):
                    ps = ps1.tile([128, 512], F32, tag="ps1")
                    for c in range(NC):
                        nc.tensor.matmul(
                            ps[:, 0:b - a],
                            wq_sb[:, c, m * 128:(m + 1) * 128],
                            x1t[:, c, a:b],
                            start=(c == 0), stop=(c == NC - 1))
                    nc.vector.tensor_scalar_add(
                        dest[:, f, a:b], ps[:, 0:b - a], bqt[:, m:m + 1])

        def emit_v(trange):
            for t in trange:
                for (n0, n1) in ((0, 512), (512, 768)):
                    ps = ps1.tile([128, 512], F32, tag="ps1")
                    for c in range(NC):
                        nc.tensor.matmul(
                            ps[:, 0:n1 - n0],
                            x1t[:, c, t * 128:(t + 1) * 128],
                            wq_sb[:, c, 2 * C + n0:2 * C + n1],
                            start=(c == 0), stop=(c == NC - 1))
                    h0, h1 = n0 // D, n1 // D
                    nc.vector.tensor_tensor(
                        vp[:, t, h0:h1, 1:D + 1],
                        ps[:, 0:n1 - n0].rearrange("p (h d) -> p h d", d=D),
                        bb[:, n0:n1].rearrange("p (h d) -> p h d", d=D),
                        OP.add)

        # PSUM is statically partitioned per pool lifetime: phase 1 runs
        # with ps1 4KB + strip pool 12KB; the O psum pool opens only after
        # the phase-1 pools close (16KB/partition budget)
        pts = {}
        ops = None

        def emit_s(f):
            """S^T strips + exp + diag triangle zeroing for head pair f."""
            pt = ptp.tile([128, 2, PTW], BF16, tag="pt", name=f"pt{f}")
            pts[f] = pt
            for grp in SGROUPS:
                g0 = grp[0]
                gw = sum(T - 128 * kt for kt in grp)
                psA = sps.tile([128, gw], F32, tag="psA")
                psB = sps.tile([128, gw], F32, tag="psB")
                o = 0
                for kt in grp:
                    # split at PSUM bank boundaries in strip coords (the
                    # q-slice of the moving operand is arbitrary); A/B
                    # interleaved so the two 64x128 row-tiles of the PE
                    # array run concurrently on hardware
                    for (s0, s1) in _chunks_512(o, o + T - 128 * kt):
                        a = kt * 128 + (s0 - o)
                        b = kt * 128 + (s1 - o)
                        for hd, ps in ((0, psA), (1, psB)):
                            p0 = 64 * hd
                            nc.tensor.matmul(
                                ps[:, s0:s1],
                                kt_[p0:p0 + 64, f, kt * 128:(kt + 1) * 128],
                                qt[p0:p0 + 64, f, a:b],
                                start=True, stop=True)
                    o += T - 128 * kt
                for hd, ps in ((0, psA), (1, psB)):
                    nc.scalar.activation(
                        pt[:, hd, OFF[g0]:OFF[g0] + gw], ps[:, 0:gw],
                        AF.Exp, bias=0.0, scale=1.0 / np.sqrt(D))
                for hd in (0, 1):
                    for kt in grp:
                        dg = pt[:, hd, OFF[kt]:OFF[kt] + 128]
                        nc.gpsimd.tensor_tensor(dg, dg, tri01[:], OP.mult)

        def emit_o(f):
            """O'^T + normalize for both heads of pair f."""
            pt = pts.pop(f)
            for hd in (0, 1):
                h = 2 * f + hd
                for (q0, q1) in ((0, 512), (512, 1024)):
                    kmax = q1 // 128
                    ps_o = ops.tile([65, 512], F32, tag="ps_o")
                    for k2 in range(kmax):
                        a = max(q0, k2 * 128)
                        rhs = pt[:, hd, OFF[k2] + a - k2 * 128:
                                 OFF[k2] + q1 - k2 * 128]
                        nc.tensor.matmul(
                            ps_o[:, a - q0:q1 - q0],
                            vp[:, k2, h, :], rhs,
                            start=(k2 == 0), stop=(k2 == kmax - 1))
                    w = q1 - q0
                    # denominator on PSUM partition 0 -> reciprocal into
                    # SBUF partition 0 -> gpsimd broadcast (SBUF-only) ->
                    # DVE multiply -> DMA shift to the head's partitions
                    dn0 = nrm.tile([1, 512], F32, tag="dn0")
                    nc.vector.reciprocal(dn0[0:1, 0:w], ps_o[0:1, 0:w])
                    bc = nrm.tile([128, 512], F32, tag="bc")
                    nc.gpsimd.partition_broadcast(bc[:, 0:w],
                                                  dn0[0:1, 0:w])
                    o_n = nrm.tile([65, 512], BF16, tag="o_n")
                    # partition 0 computes den*(1/den) - ignored; DVE ops
                    # need 32-aligned start partitions
                    nc.vector.tensor_tensor(
                        o_n[0:65, 0:w], ps_o[0:65, 0:w],
                        bc[0:65, 0:w], OP.mult)
                    deng = nc.sync if hd == 0 else nc.gpsimd
                    deng.dma_start(
                        attnT[64 * hd:64 * hd + 64, f, q0:q1],
                        o_n[1:65, 0:w])

        # interleaved emission: ACT's exp stream starts right after the
        # first Q/K pair, V fills the gap while ACT works, O after V
        emit_qk(0)
        emit_qk(1)
        emit_s(0)
        emit_qk(2)
        emit_qk(3)
        emit_s(1)
        emit_qk(4)
        emit_qk(5)
        emit_s(2)
        emit_v(range(0, 4))
        emit_s(3)
        emit_v(range(4, 8))
        ps1_cm.__exit__(None, None, None)
        p1w_cm.__exit__(None, None, None)
        ops_cm = tc.tile_pool(name="ops", bufs=4, space="PSUM")
        ops = ops_cm.__enter__()
        emit_s(4)
        emit_o(0)
        emit_o(1)
        emit_s(5)
        emit_o(2)
        emit_o(3)
        emit_o(4)
        emit_o(5)
        ops_cm.__exit__(None, None, None)
        sps_cm.__exit__(None, None, None)

        # ---------------- phase 3: projection ----------------
        with tc.tile_pool(name="p3", bufs=3) as p3, \
             tc.tile_pool(name="ps3", bufs=4, space="PSUM") as ps3:
            for t in range(NT):
                for (n0, n1) in ((0, 512), (512, 768)):
                    ps_y = ps3.tile([128, n1 - n0], F32, tag="ps_y")
                    for c in range(NC):
                        nc.tensor.matmul(
                            ps_y[:], attnT[:, c, t * 128:(t + 1) * 128],
                            wpp[:, c, n0:n1],
                            start=(c == 0), stop=(c == NC - 1))
                    y_sb = p3.tile([128, n1 - n0], F32, tag="y_sb")
                    nc.vector.tensor_tensor(y_sb[:], ps_y[:],
                                            bb[:, C + n0:C + n1], OP.add)
                    deng = nc.sync if (t + (n0 > 0)) % 2 == 0 else nc.gpsimd
                    deng.dma_start(io["y"][t * 128:(t + 1) * 128, n0:n1],
                                   y_sb[:])


def build_io(nc):
    return {
        "xT": nc.dram_tensor("xT", [C, T], BF16, kind="ExternalInput").ap(),
        "wqkv": nc.dram_tensor("wqkv", [C, 3 * C], BF16,
                               kind="ExternalInput").ap(),
        "wp": nc.dram_tensor("wp", [C, C], BF16, kind="ExternalInput").ap(),
        "bqkvT": nc.dram_tensor("bqkvT", [128, 2 * C // 128], F32,
                                kind="ExternalInput").ap(),
        "bias_bcast": nc.dram_tensor("bias_bcast", [128, 2 * C], F32,
                                     kind="ExternalInput").ap(),
        "ones": nc.dram_tensor("ones", [128, 128], F32,
                               kind="ExternalInput").ap(),
        "tri01": nc.dram_tensor("tri01", [128, 128], BF16,
                                kind="ExternalInput").ap(),
        "y": nc.dram_tensor("y", [T, C], F32, kind="ExternalOutput").ap(),
    }


def build_nc():
    from concourse import bacc
    import concourse.tile as tile
    nc = bacc.Bacc("TRN2", target_bir_lowering=False, debug=False,
                   enable_asserts=True, num_devices=N_CORES)
    io = build_io(nc)
    with tile.TileContext(nc) as tc:
        _emit_attention(tc, io)
    nc.compile()
    return nc


def host_consts():
    import ml_dtypes
    # tri01[k, j] = 1 if k <= j else 0: valid (q >= k) part of a diagonal
    # 128x128 block of S^T
    tri01 = np.triu(np.ones((128, 128), dtype=np.float32))
    return {
        "ones": np.ones((128, 128), dtype=np.float32),
        "tri01": tri01.astype(ml_dtypes.bfloat16),
    }


_NC_CACHE = None


def _get_nc():
    global _NC_CACHE
    if _NC_CACHE is None:
        _NC_CACHE = build_nc()
    return _NC_CACHE


def make_in_maps(x, c_attn_kernel, c_attn_bias, c_proj_kernel, c_proj_bias):
    import ml_dtypes
    bf = ml_dtypes.bfloat16
    consts = host_consts()
    wqkv = np.ascontiguousarray(c_attn_kernel).astype(bf)
    bqkv = np.ascontiguousarray(c_attn_bias, dtype=np.float32)
    bqkvT = np.ascontiguousarray(bqkv[0:2 * C].reshape(2 * C // 128, 128).T)
    wp = np.ascontiguousarray(c_proj_kernel).astype(bf)
    bp = np.ascontiguousarray(c_proj_bias, dtype=np.float32)
    bias_bcast = np.ascontiguousarray(
        np.tile(np.concatenate([bqkv[2 * C:], bp]), (128, 1)))
    in_maps = []
    for bb in range(N_CORES):
        m = {"xT": np.ascontiguousarray(np.asarray(x[bb]).T).astype(bf),
             "wqkv": wqkv, "bqkvT": bqkvT, "wp": wp,
             "bias_bcast": bias_bcast}
        m.update(consts)
        in_maps.append(m)
    return in_maps


def kernel(x, c_attn_kernel, c_attn_bias, c_proj_kernel, c_proj_bias):
    from concourse.bass_utils import run_bass_kernel_spmd
    x = np.asarray(x)
    assert x.shape == (B, T, C), x.shape
    nc = _get_nc()
    in_maps = make_in_maps(x, c_attn_kernel, c_attn_bias, c_proj_kernel,
                           c_proj_bias)
    res = run_bass_kernel_spmd(nc, in_maps, core_ids=list(range(N_CORES)))
    y = np.stack([res.results[bb]["y"]
                  for bb in range(N_CORES)]).astype(np.float32)
    return y


# revision 40
# speedup vs baseline: 1.8726x; 1.8726x over previous
"""Causal self-attention (B=8, T=1024, C=768, H=12) on 8 Trainium2 NeuronCores.

Sharding: data parallel - one batch element per core, no collectives.

v3: all matmuls bf16 (1 cyc/row at any free size, half the DMA/SBUF of
fp32), fp32 accumulation in PSUM.
  Q^T, K^T = Wqkv^T-tiles.T @ x^T       (features on partitions, bf16 out)
  V  = x^T-tiles.T @ Wv (+ ones col)    (natural layout per head)
  per head pair (even head on partitions 0-63, odd on 64-127; the S
  matmuls are 64-contraction row-tiled pairs emitted interleaved so both
  halves of the PE array can run concurrently on hardware):
    S^T strips = K_h^T.T @ Q_h^T        (exact causal chunks, grouped to
       cut ACT op count; pieces split at PSUM bank boundaries)
    ACT exp(s/8) -> ragged P^T (bf16); no mask matmuls - the invalid
       lower triangle of each diagonal 128x128 block is zeroed after the
       exp by a DVE multiply with a 0/1 triangle (exp of junk is finite,
       then exactly zeroed)
    O'^T = [V_h | ones64].T @ P^T       (partitions 0-63 = O^T, 64-127 =
       the softmax denominator replicated - the partition broadcast is
       free in the matmul)
    normalize: one DVE divide (in0 partitions 0-63, in1 64-127) writing
       each head's attnT half directly
  y = attn^T-tiles.T @ Wp + bias (f32)
S-strip emission is interleaved into phase 1 (after each pair's Q/K
tiles complete) so the scalar engine's exp stream - the phase-2 pacer -
starts ~15us into the kernel. No GPSIMD anywhere: its per-op dispatch
and software-DGE DMA path are far slower on hardware than the cost
model suggests. DMAs only on the SP/ACT hardware-DGE queues.
"""
import sys
from contextlib import ExitStack

import numpy as np

for _p in ("/opt/trn_rl_repo", "/root/.axon_site/_ro/trn_rl_repo"):
    if _p not in sys.path:
        sys.path.insert(0, _p)

import concourse.bass as bass  # noqa: E402
import concourse.mybir as mybir  # noqa: E402

F32 = mybir.dt.float32
BF16 = mybir.dt.bfloat16
AF = mybir.ActivationFunctionType
OP = mybir.AluOpType

B, T, C, H, D = 8, 1024, 768, 12, 64
N_CORES = 8
NT = T // 128   # 8 query/key tiles
NC = C // 128   # 6 feature tiles
NPAIR = H // 2  # 6 head pairs == feature tiles

# ragged P^T offsets: strip kt holds q in [kt*128, T)
OFF = [0] * (NT + 1)
for _i in range(NT):
    OFF[_i + 1] = OFF[_i] + (T - 128 * _i)
PTW = OFF[NT]

# PSUM strip groups: list of (kt members,) merged into one tile / one ACT.
# Max tile 1024 f32 = 2 banks, so strip pool = 4 banks and the O pool can
# quad-buffer.
SGROUPS = [(0,), (1,), (2,), (3,), (4, 5), (6, 7)]


def _chunks_512(a, b):
    out = []
    while a < b:
        nxt = min((a // 512 + 1) * 512, b)
        out.append((a, nxt))
        a = nxt
    return out


def _emit_attention(tc, io):
    nc = tc.nc

    with ExitStack() as stack:
        consts = stack.enter_context(tc.tile_pool(name="consts", bufs=1))
        persist = stack.enter_context(tc.tile_pool(name="persist", bufs=1))

        qt = persist.tile([128, NC, T], BF16, tag="qt")
        kt_ = persist.tile([128, NC, T], BF16, tag="kt")
        # per (kt, head): [V_h | ones64] so the O' matmul emits O^T on
        # partitions 0-63 AND the softmax denominator replicated on
        # partitions 64-127 - the partition broadcast comes free with the
        # same streamed rows, and normalize is a single DVE divide
        vp = persist.tile([128, NT, H, 2 * D], BF16, tag="vp")
        attnT = persist.tile([128, NC, T], BF16, tag="attnT")
        wpp = persist.tile([128, NC, C], BF16, tag="wpp")

        # phase-2 long-lived pools first (pool closes must be LIFO, and the
        # phase-1 pools close mid-emission)
        ptp = stack.enter_context(tc.tile_pool(name="pt", bufs=2))
        nrm = stack.enter_context(tc.tile_pool(name="nrm", bufs=3))
        sps_cm = tc.tile_pool(name="sps", bufs=1, space="PSUM")
        sps = sps_cm.__enter__()

        # phase-1 working set (closed mid-emission, after V)
        p1w_cm = tc.tile_pool(name="p1w", bufs=1)
        p1w = p1w_cm.__enter__()
        ps1_cm = tc.tile_pool(name="ps1", bufs=2, space="PSUM")
        ps1 = ps1_cm.__enter__()
        x1t = p1w.tile([128, NC, T], BF16, tag="x1t")
        wq_sb = p1w.tile([128, NC, 3 * C], BF16, tag="wq")

        # x first (gates first matmul), split across SP/DVE queues; weights
        # on the Pool/ACT queues so SP's serial descriptor generation
        # doesn't starve PE
        for c in range(NC):
            for hi, (a, b) in enumerate(((0, 512), (512, 1024))):
                eng = nc.sync if (c + hi) % 2 == 0 else nc.scalar
                eng.dma_start(x1t[:, c, a:b],
                              io["xT"][c * 128:(c + 1) * 128, a:b])
        tri01 = consts.tile([128, 128], BF16, tag="tri01")
        nc.scalar.dma_start(tri01[:], io["tri01"])
        bqt = consts.tile([128, 2 * C // 128], F32, tag="bqt")
        nc.scalar.dma_start(bqt[:], io["bqkvT"])
        bb = consts.tile([128, 2 * C], F32, tag="bb")
        nc.scalar.dma_start(bb[:], io["bias_bcast"])

        # Q/K weight columns in 256-wide (f, f+NC)-paired groups so the
        # m-order below unblocks in emission order; V columns after
        # weight DMAs split across the two hardware-DGE queues (SP gets Q,
        # ACT gets K - ACT's exp stream only starts ~15us in)
        for g in range(3):
            for bi, base in enumerate((0, C)):
                c0 = base + g * 256
                eng = nc.sync if bi == 0 else nc.scalar
                for c in range(NC):
                    eng.dma_start(
                        wq_sb[:, c, c0:c0 + 256],
                        io["wqkv"][c * 128:(c + 1) * 128, c0:c0 + 256])
        for c in range(NC):
            nc.sync.dma_start(wq_sb[:, c, 2 * C:3 * C],
                              io["wqkv"][c * 128:(c + 1) * 128, 2 * C:3 * C])
        for c in range(NC):
            nc.sync.dma_start(wpp[:, c, :], io["wp"][c * 128:(c + 1) * 128, :])

        # ones half of vp (columns D..2D of every head)
        nc.vector.memset(vp[:, :, :, D:2 * D], 1.0)

        def emit_qk(f):
            for m in (f, NC + f):
                dest = qt if m < NC else kt_
                for (a, b) in ((0, 512), (512, 1024)):
                    ps = ps1.tile([128, 512], F32, tag="ps1")
                    for c in range(NC):
                        nc.tensor.matmul(
                            ps[:, 0:b - a],
                            wq_sb[:, c, m * 128:(m + 1) * 128],
                            x1t[:, c, a:b],
                            start=(c == 0), stop=(c == NC - 1))
                    nc.vector.tensor_scalar_add(
                        dest[:, f, a:b], ps[:, 0:b - a], bqt[:, m:m + 1])

        def emit_v(trange):
            for t in trange:
                for (n0, n1) in ((0, 512), (512, 768)):
                    ps = ps1.tile([128, 512], F32, tag="ps1")
                    for c in range(NC):
                        nc.tensor.matmul(
                            ps[:, 0:n1 - n0],
                            x1t[:, c, t * 128:(t + 1) * 128],
                            wq_sb[:, c, 2 * C + n0:2 * C + n1],
                            start=(c == 0), stop=(c == NC - 1))
                    h0, h1 = n0 // D, n1 // D
                    nc.vector.tensor_tensor(
                        vp[:, t, h0:h1, 0:D],
                        ps[:, 0:n1 - n0].rearrange("p (h d) -> p h d", d=D),
                        bb[:, n0:n1].rearrange("p (h d) -> p h d", d=D),
                        OP.add)

        # PSUM is statically partitioned per pool lifetime: phase 1 runs
        # with ps1 4KB + strip pool 12KB; the O psum pool opens only after
        # the phase-1 pools close (16KB/partition budget)
        pts = {}
        ops = None

        def emit_s(f):
            """S^T strips + exp + diag triangle zeroing for head pair f."""
            pt = ptp.tile([128, 2, PTW], BF16, tag="pt", name=f"pt{f}")
            pts[f] = pt
            for grp in SGROUPS:
                g0 = grp[0]
                gw = sum(T - 128 * kt for kt in grp)
                psA = sps.tile([128, gw], F32, tag="psA")
                psB = sps.tile([128, gw], F32, tag="psB")
                o = 0
                for kt in grp:
                    # split at PSUM bank boundaries in strip coords (the
                    # q-slice of the moving operand is arbitrary); A/B
                    # interleaved so the two 64x128 row-tiles of the PE
                    # array run concurrently on hardware
                    for (s0, s1) in _chunks_512(o, o + T - 128 * kt):
                        a = kt * 128 + (s0 - o)
                        b = kt * 128 + (s1 - o)
                        for hd, ps in ((0, psA), (1, psB)):
                            p0 = 64 * hd
                            nc.tensor.matmul(
                                ps[:, s0:s1],
                                kt_[p0:p0 + 64, f, kt * 128:(kt + 1) * 128],
                                qt[p0:p0 + 64, f, a:b],
                                start=True, stop=True)
                    o += T - 128 * kt
                for hd, ps in ((0, psA), (1, psB)):
                    nc.scalar.activation(
                        pt[:, hd, OFF[g0]:OFF[g0] + gw], ps[:, 0:gw],
                        AF.Exp, bias=0.0, scale=1.0 / np.sqrt(D))
                for hd in (0, 1):
                    for kt in grp:
                        dg = pt[:, hd, OFF[kt]:OFF[kt] + 128]
                        nc.vector.tensor_tensor(dg, dg, tri01[:], OP.mult)

        def emit_o(f):
            """O'^T + normalize for both heads of pair f."""
            pt = pts.pop(f)
            for hd in (0, 1):
                h = 2 * f + hd
                for (q0, q1) in ((0, 512), (512, 1024)):
                    kmax = q1 // 128
                    ps_o = ops.tile([128, 512], F32, tag="ps_o")
                    for k2 in range(kmax):
                        a = max(q0, k2 * 128)
                        rhs = pt[:, hd, OFF[k2] + a - k2 * 128:
                                 OFF[k2] + q1 - k2 * 128]
                        nc.tensor.matmul(
                            ps_o[:, a - q0:q1 - q0],
                            vp[:, k2, h, :], rhs,
                            start=(k2 == 0), stop=(k2 == kmax - 1))
                    w = q1 - q0
                    # O^T on partitions 0-63, denominator replicated on
                    # 64-127 (DVE reads at most one PSUM operand, so the
                    # reciprocal hops through SBUF)
                    rec = nrm.tile([128, 512], F32, tag="rec")
                    nc.vector.reciprocal(rec[64:128, 0:w], ps_o[64:128, 0:w])
                    nc.vector.tensor_tensor(
                        attnT[64 * hd:64 * hd + 64, f, q0:q1],
                        ps_o[0:64, 0:w], rec[64:128, 0:w], OP.mult)

        # interleaved emission: ACT's exp stream starts right after the
        # first Q/K pair, V fills the gap while ACT works, O after V
        emit_qk(0)
        emit_qk(1)
        emit_s(0)
        emit_qk(2)
        emit_qk(3)
        emit_s(1)
        emit_qk(4)
        emit_qk(5)
        emit_s(2)
        emit_v(range(0, 4))
        emit_s(3)
        emit_v(range(4, 8))
        ps1_cm.__exit__(None, None, None)
        p1w_cm.__exit__(None, None, None)
        ops_cm = tc.tile_pool(name="ops", bufs=4, space="PSUM")
        ops = ops_cm.__enter__()
        emit_s(4)
        emit_o(0)
        emit_o(1)
        emit_s(5)
        emit_o(2)
        emit_o(3)
        emit_o(4)
        emit_o(5)
        ops_cm.__exit__(None, None, None)
        sps_cm.__exit__(None, None, None)

        # ---------------- phase 3: projection ----------------
        with tc.tile_pool(name="p3", bufs=3) as p3, \
             tc.tile_pool(name="ps3", bufs=4, space="PSUM") as ps3:
            for t in range(NT):
                for (n0, n1) in ((0, 512), (512, 768)):
                    ps_y = ps3.tile([128, n1 - n0], F32, tag="ps_y")
                    for c in range(NC):
                        nc.tensor.matmul(
                            ps_y[:], attnT[:, c, t * 128:(t + 1) * 128],
                            wpp[:, c, n0:n1],
                            start=(c == 0), stop=(c == NC - 1))
                    y_sb = p3.tile([128, n1 - n0], F32, tag="y_sb")
                    nc.vector.tensor_tensor(y_sb[:], ps_y[:],
                                            bb[:, C + n0:C + n1], OP.add)
                    deng = nc.sync if (t + (n0 > 0)) % 2 == 0 else nc.scalar
                    deng.dma_start(io["y"][t * 128:(t + 1) * 128, n0:n1],
                                   y_sb[:])


def build_io(nc):
    return {
        "xT": nc.dram_tensor("xT", [C, T], BF16, kind="ExternalInput").ap(),
        "wqkv": nc.dram_tensor("wqkv", [C, 3 * C], BF16,
                               kind="ExternalInput").ap(),
        "wp": nc.dram_tensor("wp", [C, C], BF16, kind="ExternalInput").ap(),
        "bqkvT": nc.dram_tensor("bqkvT", [128, 2 * C // 128], F32,
                                kind="ExternalInput").ap(),
        "bias_bcast": nc.dram_tensor("bias_bcast", [128, 2 * C], F32,
                                     kind="ExternalInput").ap(),
        "tri01": nc.dram_tensor("tri01", [128, 128], BF16,
                                kind="ExternalInput").ap(),
        "y": nc.dram_tensor("y", [T, C], F32, kind="ExternalOutput").ap(),
    }


def build_nc():
    from concourse import bacc
    import concourse.tile as tile
    nc = bacc.Bacc("TRN2", target_bir_lowering=False, debug=False,
                   enable_asserts=True, num_devices=N_CORES)
    io = build_io(nc)
    with tile.TileContext(nc) as tc:
        _emit_attention(tc, io)
    nc.compile()
    return nc


def host_consts():
    import ml_dtypes
    # tri01[k, j] = 1 if k <= j else 0: valid (q >= k) part of a diagonal
    # 128x128 block of S^T
    tri01 = np.triu(np.ones((128, 128), dtype=np.float32))
    return {
        "tri01": tri01.astype(ml_dtypes.bfloat16),
    }


_NC_CACHE = None


def _get_nc():
    global _NC_CACHE
    if _NC_CACHE is None:
        _NC_CACHE = build_nc()
    return _NC_CACHE


def make_in_maps(x, c_attn_kernel, c_attn_bias, c_proj_kernel, c_proj_bias):
    import ml_dtypes
    bf = ml_dtypes.bfloat16
    consts = host_consts()
    wqkv = np.ascontiguousarray(c_attn_kernel).astype(bf)
    bqkv = np.ascontiguousarray(c_attn_bias, dtype=np.float32)
    bqkvT = np.ascontiguousarray(bqkv[0:2 * C].reshape(2 * C // 128, 128).T)
    wp = np.ascontiguousarray(c_proj_kernel).astype(bf)
    bp = np.ascontiguousarray(c_proj_bias, dtype=np.float32)
    bias_bcast = np.ascontiguousarray(
        np.tile(np.concatenate([bqkv[2 * C:], bp]), (128, 1)))
    in_maps = []
    for bb in range(N_CORES):
        m = {"xT": np.ascontiguousarray(np.asarray(x[bb]).T).astype(bf),
             "wqkv": wqkv, "bqkvT": bqkvT, "wp": wp,
             "bias_bcast": bias_bcast}
        m.update(consts)
        in_maps.append(m)
    return in_maps


def kernel(x, c_attn_kernel, c_attn_bias, c_proj_kernel, c_proj_bias):
    from concourse.bass_utils import run_bass_kernel_spmd
    x = np.asarray(x)
    assert x.shape == (B, T, C), x.shape
    nc = _get_nc()
    in_maps = make_in_maps(x, c_attn_kernel, c_attn_bias, c_proj_kernel,
                           c_proj_bias)
    res = run_bass_kernel_spmd(nc, in_maps, core_ids=list(range(N_CORES)))
    y = np.stack([res.results[bb]["y"]
                  for bb in range(N_CORES)]).astype(np.float32)
    return y
